# revision 26
# baseline (speedup 1.0000x reference)
"""GCN encoder (GCNConv + PReLU) as a Bass/Tile kernel on 8 Trainium2 NeuronCores.

Math (matches PyG GCNConv with self-loops + symmetric norm, then PReLU):
    deg[i]  = in-degree of i over dst (+1 self loop)
    dinv    = 1/sqrt(deg)
    agg[d]  = sum_{e:(s->d)} dinv[s]*dinv[d] * x[s] + dinv[d]^2 * x[d]
    out     = PReLU(agg @ W.T + bias)

Distribution: dst-node sharding, core k owns nodes [k*6250, (k+1)*6250).

Key structure (v2):
  - x is host-prescaled by dinv[src] and stored bf16 in two 25000-row halves
    (int16 gather indices). Per dst-group (GBLK blocks of 128 dst), edges are
    packed contiguously (block-major) and fetched with dma_gather; trailing
    index slots are -1 so the Q7 descriptor generator trims them (desc-gen
    cost == real edge count, no cross-core padding cost).
  - Msel[e, d] = (iota == dstl[e]) is a pure 0/1 selection built with a single
    DVE is_equal op (bf16), optionally some on ACT via relu(1 - |iota-dstl|).
  - The scatter-add accumulates the TRANSPOSED aggregate directly:
        AT_h[c, d] += gx[e, c]^T @ Msel[e, d]   (PE, bf16, PSUM f32)
    so no PE transpose pass is needed before the weight matmul.
  - Self-loop: AT_h += xsall[d, c]^T @ I via one identity matmul per half
    (xsall = dinv * x own rows, dense HWDGE load).
  - H[d, :] = sum_h AT_h^T @ W_h + sqrtdeg[d] * bias  (PSUM accumulation;
    the sqrtdeg row makes the bias exact after the final dinv[d] scaling).
  - out = Prelu(H * dinv[d]) in ONE scalar-engine activation (parametric_relu
    is resident in every activation table; alpha is the PReLU scalar).

Env knobs: GCN_GBLK (2), GCN_NBUF (8), GCN_MSACT (0..8: of every 8 msel
builds, this many go to ACT), GCN_PRELU=act|max.
"""

import os
import numpy as np

import concourse.bass as bass
import concourse.tile as tile
from concourse import bacc, mybir, bass_utils
from contextlib import ExitStack

# Problem shape (fixed by the harness contract).
N_NODES = 50000
N_EDGES = 400000
IN_CH = 256
HID = 512
NCORES = 8
NPC = N_NODES // NCORES  # dst nodes owned per core
P = 128
BPC = (NPC + P - 1) // P  # dst blocks per core (49)

F32 = mybir.dt.float32
F32R = mybir.dt.float32r
BF16 = mybir.dt.bfloat16

GBLK = int(os.environ.get("GCN_GBLK", "2"))
FULLGEN = os.environ.get("GCN_FULLGEN", "0") == "1"
NBUF = int(os.environ.get("GCN_NBUF", "8"))
MSACT = int(os.environ.get("GCN_MSACT", "0"))
PRELU_MODE = os.environ.get("GCN_PRELU", "act")
MS_DT = os.environ.get("GCN_MS_DT", "fp8")  # fp8 | bf16
OUT_DT = os.environ.get("GCN_OUT_DT", "bf16")  # bf16 | f32


def _preprocess(edge_index, n_nodes=N_NODES, ncores=NCORES, gblk=GBLK, nbuf=NBUF):
    """Group non-self edges by (core, dst-group, src-half), pack block-major.

    Returns (sched, kmax, npairs, idx16, dstl, cnts, dinv):
      sched: per group g: dict(kk=[Klo,Khi] static chunk counts,
             nidx=[...] static gather num_idxs,
             mm={h: [(j, local_b, paircol), ...]} static matmul schedule)
      idx16: [ncores, 128, 2*ngroups*8*kmax] int16 gather indices
      dstl:  [ncores, 128, npairs] f32; dst-local in block or -1
      dinv:  [n_nodes] f32
    """
    npc = n_nodes // ncores
    half = n_nodes // 2
    ngroups = (BPC + gblk - 1) // gblk
    src = np.asarray(edge_index[0]).astype(np.int64).ravel()
    dst = np.asarray(edge_index[1]).astype(np.int64).ravel()
    deg = np.bincount(dst, minlength=n_nodes).astype(np.float32) + 1.0
    dinv = (1.0 / np.sqrt(deg)).astype(np.float32)

    core = dst // npc
    dloc = dst - core * npc
    blk = dloc // P
    grp = blk // gblk
    hi = (src >= half).astype(np.int64)

    # sort edges by (core, grp, hi, blk, src)
    key = (((core * ngroups + grp) * 2 + hi) * BPC + blk) * (half + 1) + (
        src - hi * half
    )
    order = np.argsort(key, kind="stable")
    src_s, core_s = src[order], core[order]
    grp_s, hi_s, blk_s = grp[order], hi[order], blk[order]
    dll_s = (dloc[order] - blk_s * P).astype(np.int64)  # dst local in block

    # counts per (core, grp, hi) and per (core, grp, hi, blk)
    cgh = (core_s * ngroups + grp_s) * 2 + hi_s
    n_cgh = ncores * ngroups * 2
    cnt_cgh = np.bincount(cgh, minlength=n_cgh).reshape(ncores, ngroups, 2)
    cghb = cgh * BPC + blk_s
    cnt_cghb = np.bincount(cghb, minlength=n_cgh * BPC).reshape(
        ncores, ngroups, 2, BPC
    )

    kgh = -(-cnt_cgh.max(axis=0) // P)  # [ngroups, 2] static chunk counts
    kgh = np.maximum(kgh, 1)
    kmax = int(kgh.max())

    # rank of each edge within its (core, grp, hi) segment
    seg_start = np.zeros(n_cgh + 1, np.int64)
    seg_start[1:] = np.cumsum(cnt_cgh.ravel())
    rank = np.arange(len(order)) - seg_start[cgh]

    # static matmul schedule: union over cores of blocks present in chunk j
    # block b of (g,h) occupies ranks [bs, bs+cnt) -> chunks bs//P .. (bs+cnt-1)//P
    bstart = np.cumsum(cnt_cghb, axis=3) - cnt_cghb  # [nc, ng, 2, BPC]
    sched = []
    npairs = 0
    paircol = {}
    for g in range(ngroups):
        blocks = list(range(g * gblk, min((g + 1) * gblk, BPC)))
        mm = {0: [], 1: []}
        for h in range(2):
            k = int(kgh[g, h])
            for j in range(k):
                for b in blocks:
                    bl = b - g * gblk
                    lo = bstart[:, g, h, b]
                    cnt = cnt_cghb[:, g, h, b]
                    # does block b intersect chunk j on any core?
                    inter = np.any(
                        (cnt > 0) & (lo < (j + 1) * P) & (lo + cnt > j * P)
                    )
                    if inter:
                        paircol[(g, h, j, b)] = npairs
                        mm[h].append((j, bl, npairs))
                        npairs += 1
        nidx = [
            (kmax if g < nbuf else int(kgh[g, h])) * P for h in range(2)
        ]
        pc1 = npairs
        sched.append(
            {"kk": [int(kgh[g, 0]), int(kgh[g, 1])], "nidx": nidx, "mm": mm,
             "pc0": pc1 - len(mm[0]) - len(mm[1]), "pc1": pc1}
        )

    # per-core data arrays
    segw = 8 * kmax  # idx16 columns per (g,h) segment
    fill = 0 if FULLGEN else -1
    idx16 = np.full((ncores, 16, 2 * ngroups * segw), fill, np.int16)
    # first-rotation groups: pad with 0 (valid) so gx pool buffers are
    # fully written once before trimmed gathers leave stale tails
    for g in range(min(nbuf, ngroups)):
        for h in range(2):
            s = (g * 2 + h) * segw
            idx16[:, :, s : s + segw] = 0
    seg = (grp_s * 2 + hi_s) * segw
    col = seg + (rank // 16)
    row = rank % 16
    idx16[core_s, row, col] = (src_s - hi_s * half).astype(np.int16)

    # msel one-hot tiles: edge at rank r -> chunk j=r//P, slot p=r%P;
    # pair (g,h,j,blk) -> column pc*P + dst_local
    j_s = rank // P
    p_s = rank % P
    pc = np.array(
        [paircol[(g, h, j, b)] for g, h, j, b in zip(grp_s, hi_s, j_s, blk_s)],
        dtype=np.int64,
    )
    msel = np.zeros((ncores, P, npairs * P), np.float32)
    msel[core_s, p_s, pc * P + dll_s] = 1.0

    idx16 = np.tile(idx16, (1, 8, 1))  # replicate to 128 partitions

    # runtime gather counts per (core, g, h): real edges (trailing -1 slots
    # are trimmed by the Q7 desc generator); first-rotation groups gather
    # their full static extent (padded with index 0)
    cnts = np.empty((ncores, 2 * ngroups), np.int32)
    for g in range(ngroups):
        for h in range(2):
            if g < nbuf:
                cnts[:, g * 2 + h] = kmax * P
            elif FULLGEN:
                cnts[:, g * 2 + h] = int(
                    np.ceil(cnt_cgh[:, g, h].max() / P)
                ) * P
            else:
                cnts[:, g * 2 + h] = cnt_cgh[:, g, h]
    return sched, kmax, npairs, idx16, msel, cnts, dinv


def _build_program(
    sched,
    kmax,
    npairs,
    alpha,
    n_nodes=N_NODES,
    ncores=NCORES,
    in_ch=IN_CH,
    hid=HID,
    gblk=GBLK,
    nbuf=NBUF,
    msact=MSACT,
    prelu_mode=PRELU_MODE,
):
    npc = n_nodes // ncores
    ngroups = len(sched)
    segw = 8 * kmax
    nch = in_ch // P  # 2 channel halves

    nc = bacc.Bacc(
        "TRN2", target_bir_lowering=False, debug=False,
        num_swdge_queues=4,
        dynamic_dma_scratch_size=int(os.environ.get("GCN_SCRATCH", "65536")),
    )
    half = n_nodes // 2
    x_ds = [
        nc.dram_tensor(f"x{h}", [half, in_ch], BF16, kind="ExternalInput")
        for h in range(2)
    ]
    si_d = nc.dram_tensor(
        "idx16", [P, 2 * ngroups * segw], mybir.dt.int16, kind="ExternalInput"
    )
    ms_dt = mybir.dt.float8e4 if MS_DT == "fp8" else BF16
    out_dt = BF16 if OUT_DT == "bf16" else F32
    ms_d = nc.dram_tensor("msel", [P, npairs * P], ms_dt, kind="ExternalInput")
    xs_d = nc.dram_tensor("xsall", [P, BPC * in_ch], BF16, kind="ExternalInput")
    wt_ds = [
        nc.dram_tensor(f"wt{h}", [P, hid], BF16, kind="ExternalInput")
        for h in range(nch)
    ]
    bs_d = nc.dram_tensor("bias", [1, hid], F32R, kind="ExternalInput")
    sdg_d = nc.dram_tensor("sdg", [1, BPC * P], F32R, kind="ExternalInput")
    idr_d = nc.dram_tensor("idr", [P, P], BF16, kind="ExternalInput")
    ct_d = nc.dram_tensor(
        "cnts", [1, 2 * ngroups], mybir.dt.int32, kind="ExternalInput"
    )
    dv_d = nc.dram_tensor("dinvc", [P, BPC], F32, kind="ExternalInput")
    adv_d = nc.dram_tensor("adinvc", [P, BPC], F32, kind="ExternalInput")
    out_d = nc.dram_tensor("out", [npc, hid], out_dt, kind="ExternalOutput")

    with tile.TileContext(nc) as tc, ExitStack() as ctx:
        const = ctx.enter_context(tc.tile_pool(name="const", bufs=1))
        gxp = ctx.enter_context(tc.tile_pool(name="gx", bufs=nbuf))
        mselp = ctx.enter_context(tc.tile_pool(name="msel", bufs=int(os.environ.get("GCN_MSELBUF", "5"))))
        psA = ctx.enter_context(tc.tile_pool(name="psA", bufs=2, space="PSUM"))
        psT = ctx.enter_context(tc.tile_pool(name="psT", bufs=1, space="PSUM"))
        hps = ctx.enter_context(tc.tile_pool(name="hps", bufs=int(os.environ.get("GCN_HPSBUF", "2")), space="PSUM"))
        aS = ctx.enter_context(tc.tile_pool(name="aS", bufs=int(os.environ.get("GCN_ASBUF", "4"))))
        outp = ctx.enter_context(tc.tile_pool(name="outp", bufs=3))

        ct_t = const.tile([1, 2 * ngroups], mybir.dt.int32)
        nc.sync.dma_start(out=ct_t[:], in_=ct_d.ap())
        si_t = const.tile([P, 2 * ngroups * segw], mybir.dt.int16)
        si_cols = 2 * ngroups * segw
        si_step = -(-si_cols // 8)
        for c0 in range(0, si_cols, si_step):
            c1 = min(c0 + si_step, si_cols)
            nc.sync.dma_start(out=si_t[:, c0:c1], in_=si_d.ap()[:, c0:c1])
        xs_t = const.tile([P, BPC * in_ch], BF16)
        xs_cols = BPC * in_ch
        xs_step = -(-xs_cols // 8)
        for c0 in range(0, xs_cols, xs_step):
            c1 = min(c0 + xs_step, xs_cols)
            nc.scalar.dma_start(out=xs_t[:, c0:c1], in_=xs_d.ap()[:, c0:c1])
        wt_t = []
        for h in range(nch):
            w = const.tile([P, hid], BF16, name=f"wt_t{h}")
            nc.sync.dma_start(out=w[:], in_=wt_ds[h].ap())
            wt_t.append(w)
        bs_t = const.tile([1, hid], F32R)
        nc.sync.dma_start(out=bs_t[:], in_=bs_d.ap())
        sdg_t = const.tile([1, BPC * P], F32R)
        nc.sync.dma_start(out=sdg_t[:], in_=sdg_d.ap())
        idr_t = const.tile([P, P], BF16)
        nc.sync.dma_start(out=idr_t[:], in_=idr_d.ap())
        greg = nc.alloc_register(mybir.EngineType.Pool, "gcnt")
        dv_t = const.tile([P, BPC], F32)
        nc.sync.dma_start(out=dv_t[:], in_=dv_d.ap())
        adv_t = const.tile([P, BPC], F32)
        nc.sync.dma_start(out=adv_t[:], in_=adv_d.ap())

        gather_qn = 0
        for g in range(ngroups):
            sg = sched[g]
            blocks = list(range(g * gblk, min((g + 1) * gblk, BPC)))
            nbl = len(blocks)
            # gathers (lo/hi)
            gxs = []
            for h in range(2):
                nidx = sg["nidx"][h]
                gx = gxp.tile(
                    [P, kmax * in_ch], BF16, tag=f"gx{h}", name=f"gx{h}_{g}"
                )
                kk = nidx // P
                soff = (g * 2 + h) * segw
                if not FULLGEN:
                    nc.gpsimd.reg_load(
                        greg, ct_t[0:1, g * 2 + h : g * 2 + h + 1]
                    )
                nc.gpsimd.dma_gather(
                    gx[:, : kk * in_ch].rearrange("p (k d) -> p k d", d=in_ch),
                    x_ds[h].ap(),
                    si_t[:, soff : soff + 8 * kk],
                    nidx,
                    nidx if FULLGEN else greg,
                    in_ch,
                    queue_num=gather_qn % 4,
                    single_packet=False,
                )
                gather_qn += 1
                gxs.append(gx)
            # A accumulators [dst, ch] per local block (one PSUM bank each)
            Ap = {}
            started = {}
            for bl in range(nbl):
                Ap[bl] = psA.tile([P, in_ch], F32, tag=f"a{bl}", name=f"a{bl}_{g}")
                started[bl] = False
            # msel tiles for this group arrive via one HWDGE DMA
            pc0, pc1 = sg["pc0"], sg["pc1"]
            npg = pc1 - pc0
            ms_t = mselp.tile(
                [P, npg * P], ms_dt, tag="msg", name=f"msg_{g}"
            )
            nc.scalar.dma_start(
                out=ms_t[:], in_=ms_d.ap()[:, pc0 * P : pc1 * P]
            )
            # chunk matmuls: A[dst, ch] += Msel[e, dst]^T @ gx[e, ch]
            for h in range(2):
                for (j, bl, col) in sg["mm"][h]:
                    ci = col - pc0
                    gx = gxs[h]
                    nc.tensor.matmul(
                        Ap[bl][:],
                        lhsT=ms_t[:, ci * P : (ci + 1) * P],
                        rhs=gx[:, j * in_ch : (j + 1) * in_ch],
                        start=not started[bl],
                        stop=False,
                    )
                    started[bl] = True
            # per block: self-loop (stop), AT->SBUF, weight mm, PReLU, store
            for bl, b in enumerate(blocks):
                ns = min(P, npc - b * P)
                nc.tensor.matmul(
                    Ap[bl][:],
                    lhsT=idr_t[:],
                    rhs=xs_t[:, b * in_ch : (b + 1) * in_ch],
                    start=not started[bl],
                    stop=True,
                )
                started[bl] = True
                a_s = aS.tile([P, in_ch], BF16, tag="as", name=f"as_{b}")
                nc.scalar.copy(a_s[:], Ap[bl][:])
                ats = []
                for hh in range(nch):
                    pt = psT.tile(
                        [P, P], BF16, tag=f"pt{hh}", name=f"pt{hh}_{b}"
                    )
                    nc.tensor.transpose(
                        out=pt[:], in_=a_s[:, hh * P : (hh + 1) * P],
                        identity=idr_t[:],
                    )
                    a = aS.tile([P, P], BF16, tag=f"ats{hh}", name=f"ats{hh}_{b}")
                    nc.scalar.copy(a[:], pt[:])
                    ats.append(a)
                Hp = hps.tile([P, hid], F32, tag="hp", name=f"hp_{b}")
                for hh in range(nch):
                    nc.tensor.matmul(
                        Hp[:ns],
                        lhsT=ats[hh][:, :ns],
                        rhs=wt_t[hh][:],
                        start=(hh == 0),
                        stop=False,
                    )
                nc.tensor.matmul(
                    Hp[:ns],
                    lhsT=sdg_t[:, b * P : b * P + ns],
                    rhs=bs_t[:],
                    start=False,
                    stop=True,
                )
                os_ = outp.tile([P, hid], out_dt, tag="os", name=f"os_{b}")
                if prelu_mode == "act":
                    # PReLU(dinv*H) in one ACT op (exact for any alpha)
                    nc.scalar.activation(
                        out=os_[:ns],
                        in_=Hp[:ns],
                        func=mybir.ActivationFunctionType.Prelu,
                        scale=dv_t[:ns, b : b + 1],
                        alpha=float(alpha),
                    )
                else:
                    # fallback: max(dinv*H, alpha*dinv*H) (0<=alpha<=1)
                    t1 = outp.tile([P, hid], F32, tag="t1", name=f"t1_{b}")
                    t2 = outp.tile([P, hid], F32, tag="t2", name=f"t2_{b}")

                    nc.scalar.activation(
                        out=t1[:ns],
                        in_=Hp[:ns],
                        func=mybir.ActivationFunctionType.Copy,
                        scale=dv_t[:ns, b : b + 1],
                    )
                    nc.scalar.activation(
                        out=t2[:ns],
                        in_=Hp[:ns],
                        func=mybir.ActivationFunctionType.Copy,
                        scale=adv_t[:ns, b : b + 1],
                    )
                    nc.vector.tensor_tensor(
                        out=os_[:ns], in0=t1[:ns], in1=t2[:ns],
                        op=mybir.AluOpType.max,
                    )
                row0 = b * P
                nc.sync.dma_start(
                    out=out_d.ap()[row0 : row0 + ns, :], in_=os_[:ns, :]
                )
    nc.compile()
    return nc


def _make_in_maps(x, weight, bias, idx16, msel, cnts, dinv, alpha, ncores=NCORES):
    x = np.asarray(x, dtype=np.float32)
    w = np.asarray(weight, dtype=np.float32)
    n = x.shape[0]
    half = n // 2
    in_ch = x.shape[1]
    hid = w.shape[0]
    npc = n // ncores
    bf = mybir.dt.np(BF16)

    xp = x * dinv[:, None]  # prescaled by dinv[src]
    xlo = np.ascontiguousarray(xp[:half].astype(bf))
    xhi = np.ascontiguousarray(xp[half:].astype(bf))
    wts = {
        f"wt{h}": np.ascontiguousarray(w[:, h * P : (h + 1) * P].T.astype(bf))
        for h in range(in_ch // P)
    }
    bias_row = np.asarray(bias, dtype=np.float32).reshape(1, hid)
    sdeg = (1.0 / dinv).astype(np.float32)  # sqrt(deg)

    in_maps = []
    for k in range(ncores):
        sl = slice(k * npc, (k + 1) * npc)
        # xsall[d, b*256 + c] = (dinv*x)[k*npc + b*128 + d, c]
        xs = np.zeros((BPC * P, in_ch), np.float32)
        xs[:npc] = xp[sl]
        xsall = np.ascontiguousarray(
            xs.reshape(BPC, P, in_ch).transpose(1, 0, 2).reshape(P, BPC * in_ch)
        ).astype(bf)
        dv = np.zeros((BPC * P,), np.float32)
        dv[:npc] = dinv[sl]
        dvc = np.ascontiguousarray(dv.reshape(BPC, P).T)
        sdg = np.zeros((1, BPC * P), np.float32)
        sdg[0, :npc] = sdeg[sl]
        m = {
            "x0": xlo,
            "x1": xhi,
            "idx16": np.ascontiguousarray(idx16[k]),
            "cnts": np.ascontiguousarray(cnts[k : k + 1]),
            "msel": np.ascontiguousarray(
                msel[k].astype(mybir.dt.np(mybir.dt.float8e4) if MS_DT == "fp8" else bf)
            ),
            "xsall": xsall,
            "bias": bias_row,
            "sdg": sdg,
            "idr": np.eye(P, dtype=bf),
            "dinvc": dvc,
            "adinvc": np.ascontiguousarray(dvc * float(alpha)),
        }
        m.update(wts)
        in_maps.append(m)
    return in_maps


# Results of the last kernel() call, for the test harness.
LAST_RESULTS = None


def kernel(x, edge_index, weight, bias, prelu_a):
    global LAST_RESULTS
    trace = os.environ.get("GCN_TRACE", "0") == "1"

    sched, kmax, npairs, idx16, msel, cnts, dinv = _preprocess(edge_index)
    alpha = float(np.asarray(prelu_a).ravel()[0])
    nc = _build_program(sched, kmax, npairs, alpha)
    in_maps = _make_in_maps(x, weight, bias, idx16, msel, cnts, dinv, alpha)

    res = bass_utils.run_bass_kernel_spmd(
        nc, in_maps, core_ids=list(range(NCORES)), trace=trace
    )
    LAST_RESULTS = res
    out = np.concatenate(
        [np.asarray(res.results[k]["out"], dtype=np.float32) for k in range(NCORES)],
        axis=0,
    )
    return out


# revision 27
# speedup vs baseline: 1.0180x; 1.0180x over previous
"""GCN encoder (GCNConv + PReLU) as a Bass/Tile kernel on 8 Trainium2 NeuronCores.

Math (matches PyG GCNConv with self-loops + symmetric norm, then PReLU):
    deg[i]  = in-degree of i over dst (+1 self loop)
    dinv    = 1/sqrt(deg)
    agg[d]  = sum_{e:(s->d)} dinv[s]*dinv[d] * x[s] + dinv[d]^2 * x[d]
    out     = PReLU(agg @ W.T + bias)

Distribution: dst-node sharding, core k owns nodes [k*6250, (k+1)*6250).

Key structure (v2):
  - x is host-prescaled by dinv[src] and stored bf16 in two 25000-row halves
    (int16 gather indices). Per dst-group (GBLK blocks of 128 dst), edges are
    packed contiguously (block-major) and fetched with dma_gather; trailing
    index slots are -1 so the Q7 descriptor generator trims them (desc-gen
    cost == real edge count, no cross-core padding cost).
  - Msel[e, d] = (iota == dstl[e]) is a pure 0/1 selection built with a single
    DVE is_equal op (bf16), optionally some on ACT via relu(1 - |iota-dstl|).
  - The scatter-add accumulates the TRANSPOSED aggregate directly:
        AT_h[c, d] += gx[e, c]^T @ Msel[e, d]   (PE, bf16, PSUM f32)
    so no PE transpose pass is needed before the weight matmul.
  - Self-loop: AT_h += xsall[d, c]^T @ I via one identity matmul per half
    (xsall = dinv * x own rows, dense HWDGE load).
  - H[d, :] = sum_h AT_h^T @ W_h + sqrtdeg[d] * bias  (PSUM accumulation;
    the sqrtdeg row makes the bias exact after the final dinv[d] scaling).
  - out = Prelu(H * dinv[d]) in ONE scalar-engine activation (parametric_relu
    is resident in every activation table; alpha is the PReLU scalar).

Env knobs: GCN_GBLK (2), GCN_NBUF (8), GCN_MSACT (0..8: of every 8 msel
builds, this many go to ACT), GCN_PRELU=act|max.
"""

import os
import numpy as np

import concourse.bass as bass
import concourse.tile as tile
from concourse import bacc, mybir, bass_utils
from contextlib import ExitStack

# Problem shape (fixed by the harness contract).
N_NODES = 50000
N_EDGES = 400000
IN_CH = 256
HID = 512
NCORES = 8
NPC = N_NODES // NCORES  # dst nodes owned per core
P = 128
BPC = (NPC + P - 1) // P  # dst blocks per core (49)

F32 = mybir.dt.float32
F32R = mybir.dt.float32r
BF16 = mybir.dt.bfloat16

GBLK = int(os.environ.get("GCN_GBLK", "2"))
FULLGEN = os.environ.get("GCN_FULLGEN", "0") == "1"
NBUF = int(os.environ.get("GCN_NBUF", "8"))
MSACT = int(os.environ.get("GCN_MSACT", "0"))
PRELU_MODE = os.environ.get("GCN_PRELU", "act")
MS_DT = os.environ.get("GCN_MS_DT", "fp8")  # fp8 | bf16
OUT_DT = os.environ.get("GCN_OUT_DT", "bf16")  # bf16 | f32


def _preprocess(edge_index, n_nodes=N_NODES, ncores=NCORES, gblk=GBLK, nbuf=NBUF):
    """Group non-self edges by (core, dst-group, src-half), pack block-major.

    Returns (sched, kmax, npairs, idx16, dstl, cnts, dinv):
      sched: per group g: dict(kk=[Klo,Khi] static chunk counts,
             nidx=[...] static gather num_idxs,
             mm={h: [(j, local_b, paircol), ...]} static matmul schedule)
      idx16: [ncores, 128, 2*ngroups*8*kmax] int16 gather indices
      dstl:  [ncores, 128, npairs] f32; dst-local in block or -1
      dinv:  [n_nodes] f32
    """
    npc = n_nodes // ncores
    half = n_nodes // 2
    ngroups = (BPC + gblk - 1) // gblk
    src = np.asarray(edge_index[0]).astype(np.int64).ravel()
    dst = np.asarray(edge_index[1]).astype(np.int64).ravel()
    deg = np.bincount(dst, minlength=n_nodes).astype(np.float32) + 1.0
    dinv = (1.0 / np.sqrt(deg)).astype(np.float32)

    core = dst // npc
    dloc = dst - core * npc
    blk = dloc // P
    grp = blk // gblk
    hi = (src >= half).astype(np.int64)

    # sort edges by (core, grp, hi, blk, src)
    key = (((core * ngroups + grp) * 2 + hi) * BPC + blk) * (half + 1) + (
        src - hi * half
    )
    order = np.argsort(key, kind="stable")
    src_s, core_s = src[order], core[order]
    grp_s, hi_s, blk_s = grp[order], hi[order], blk[order]
    dll_s = (dloc[order] - blk_s * P).astype(np.int64)  # dst local in block

    # counts per (core, grp, hi) and per (core, grp, hi, blk)
    cgh = (core_s * ngroups + grp_s) * 2 + hi_s
    n_cgh = ncores * ngroups * 2
    cnt_cgh = np.bincount(cgh, minlength=n_cgh).reshape(ncores, ngroups, 2)
    cghb = cgh * BPC + blk_s
    cnt_cghb = np.bincount(cghb, minlength=n_cgh * BPC).reshape(
        ncores, ngroups, 2, BPC
    )

    kgh = -(-cnt_cgh.max(axis=0) // P)  # [ngroups, 2] static chunk counts
    kgh = np.maximum(kgh, 1)
    kmax = int(kgh.max())

    # rank of each edge within its (core, grp, hi) segment
    seg_start = np.zeros(n_cgh + 1, np.int64)
    seg_start[1:] = np.cumsum(cnt_cgh.ravel())
    rank = np.arange(len(order)) - seg_start[cgh]

    # static matmul schedule: union over cores of blocks present in chunk j
    # block b of (g,h) occupies ranks [bs, bs+cnt) -> chunks bs//P .. (bs+cnt-1)//P
    bstart = np.cumsum(cnt_cghb, axis=3) - cnt_cghb  # [nc, ng, 2, BPC]
    sched = []
    npairs = 0
    paircol = {}
    for g in range(ngroups):
        blocks = list(range(g * gblk, min((g + 1) * gblk, BPC)))
        mm = {0: [], 1: []}
        for h in range(2):
            k = int(kgh[g, h])
            for j in range(k):
                for b in blocks:
                    bl = b - g * gblk
                    lo = bstart[:, g, h, b]
                    cnt = cnt_cghb[:, g, h, b]
                    # does block b intersect chunk j on any core?
                    inter = np.any(
                        (cnt > 0) & (lo < (j + 1) * P) & (lo + cnt > j * P)
                    )
                    if inter:
                        paircol[(g, h, j, b)] = npairs
                        mm[h].append((j, bl, npairs))
                        npairs += 1
        nidx = [
            (kmax if g < nbuf else int(kgh[g, h])) * P for h in range(2)
        ]
        pc1 = npairs
        sched.append(
            {"kk": [int(kgh[g, 0]), int(kgh[g, 1])], "nidx": nidx, "mm": mm,
             "pc0": pc1 - len(mm[0]) - len(mm[1]), "pc1": pc1}
        )

    # per-core data arrays
    segw = 8 * kmax  # idx16 columns per (g,h) segment
    fill = 0 if FULLGEN else -1
    idx16 = np.full((ncores, 16, 2 * ngroups * segw), fill, np.int16)
    # first-rotation groups: pad with 0 (valid) so gx pool buffers are
    # fully written once before trimmed gathers leave stale tails
    for g in range(min(nbuf, ngroups)):
        for h in range(2):
            s = (g * 2 + h) * segw
            idx16[:, :, s : s + segw] = 0
    seg = (grp_s * 2 + hi_s) * segw
    col = seg + (rank // 16)
    row = rank % 16
    idx16[core_s, row, col] = (src_s - hi_s * half).astype(np.int16)

    # msel one-hot tiles: edge at rank r -> chunk j=r//P, slot p=r%P;
    # pair (g,h,j,blk) -> column pc*P + dst_local
    j_s = rank // P
    p_s = rank % P
    pc = np.array(
        [paircol[(g, h, j, b)] for g, h, j, b in zip(grp_s, hi_s, j_s, blk_s)],
        dtype=np.int64,
    )
    msel = np.zeros((ncores, P, npairs * P), np.float32)
    msel[core_s, p_s, pc * P + dll_s] = 1.0

    idx16 = np.tile(idx16, (1, 8, 1))  # replicate to 128 partitions

    # runtime gather counts per (core, g, h): real edges (trailing -1 slots
    # are trimmed by the Q7 desc generator); first-rotation groups gather
    # their full static extent (padded with index 0)
    cnts = np.empty((ncores, 2 * ngroups), np.int32)
    for g in range(ngroups):
        for h in range(2):
            if g < nbuf:
                cnts[:, g * 2 + h] = kmax * P
            elif FULLGEN:
                cnts[:, g * 2 + h] = int(
                    np.ceil(cnt_cgh[:, g, h].max() / P)
                ) * P
            else:
                cnts[:, g * 2 + h] = cnt_cgh[:, g, h]
    return sched, kmax, npairs, idx16, msel, cnts, dinv


def _build_program(
    sched,
    kmax,
    npairs,
    alpha,
    has_bias=True,
    n_nodes=N_NODES,
    ncores=NCORES,
    in_ch=IN_CH,
    hid=HID,
    gblk=GBLK,
    nbuf=NBUF,
    msact=MSACT,
    prelu_mode=PRELU_MODE,
):
    npc = n_nodes // ncores
    ngroups = len(sched)
    segw = 8 * kmax
    nch = in_ch // P  # 2 channel halves

    nc = bacc.Bacc(
        "TRN2", target_bir_lowering=False, debug=False,
        num_swdge_queues=4,
        dynamic_dma_scratch_size=int(os.environ.get("GCN_SCRATCH", "65536")),
    )
    half = n_nodes // 2
    x_ds = [
        nc.dram_tensor(f"x{h}", [half, in_ch], BF16, kind="ExternalInput")
        for h in range(2)
    ]
    si_d = nc.dram_tensor(
        "idx16", [P, 2 * ngroups * segw], mybir.dt.int16, kind="ExternalInput"
    )
    ms_dt = mybir.dt.float8e4 if MS_DT == "fp8" else BF16
    out_dt = BF16 if OUT_DT == "bf16" else F32
    ms_d = nc.dram_tensor("msel", [P, npairs * P], ms_dt, kind="ExternalInput")
    xs_d = nc.dram_tensor("xsall", [P, BPC * in_ch], BF16, kind="ExternalInput")
    wt_ds = [
        nc.dram_tensor(f"wt{h}", [P, hid], BF16, kind="ExternalInput")
        for h in range(nch)
    ]
    bs_d = nc.dram_tensor("bias", [1, hid], F32R, kind="ExternalInput")
    sdg_d = nc.dram_tensor("sdg", [1, BPC * P], F32R, kind="ExternalInput")
    idr_d = nc.dram_tensor("idr", [P, P], BF16, kind="ExternalInput")
    ct_d = nc.dram_tensor(
        "cnts", [1, 2 * ngroups], mybir.dt.int32, kind="ExternalInput"
    )
    dv_d = nc.dram_tensor("dinvc", [P, BPC], F32, kind="ExternalInput")
    adv_d = nc.dram_tensor("adinvc", [P, BPC], F32, kind="ExternalInput")
    out_d = nc.dram_tensor("out", [npc, hid], out_dt, kind="ExternalOutput")

    with tile.TileContext(nc) as tc, ExitStack() as ctx:
        const = ctx.enter_context(tc.tile_pool(name="const", bufs=1))
        gxp = ctx.enter_context(tc.tile_pool(name="gx", bufs=nbuf))
        mselp = ctx.enter_context(tc.tile_pool(name="msel", bufs=int(os.environ.get("GCN_MSELBUF", "5"))))
        psA = ctx.enter_context(tc.tile_pool(name="psA", bufs=2, space="PSUM"))
        hps = ctx.enter_context(tc.tile_pool(name="hps", bufs=int(os.environ.get("GCN_HPSBUF", "3")), space="PSUM"))
        aS = ctx.enter_context(tc.tile_pool(name="aS", bufs=int(os.environ.get("GCN_ASBUF", "4"))))
        outp = ctx.enter_context(tc.tile_pool(name="outp", bufs=3))

        ct_t = const.tile([1, 2 * ngroups], mybir.dt.int32)
        nc.sync.dma_start(out=ct_t[:], in_=ct_d.ap())
        si_t = const.tile([P, 2 * ngroups * segw], mybir.dt.int16)
        si_cols = 2 * ngroups * segw
        si_step = -(-si_cols // 8)
        for c0 in range(0, si_cols, si_step):
            c1 = min(c0 + si_step, si_cols)
            nc.sync.dma_start(out=si_t[:, c0:c1], in_=si_d.ap()[:, c0:c1])
        xs_t = const.tile([P, BPC * in_ch], BF16)
        xs_cols = BPC * in_ch
        xs_step = -(-xs_cols // 8)
        for c0 in range(0, xs_cols, xs_step):
            c1 = min(c0 + xs_step, xs_cols)
            nc.scalar.dma_start(out=xs_t[:, c0:c1], in_=xs_d.ap()[:, c0:c1])
        wt_t = []
        for h in range(nch):
            w = const.tile([P, hid], BF16, name=f"wt_t{h}")
            nc.sync.dma_start(out=w[:], in_=wt_ds[h].ap())
            wt_t.append(w)
        bs_t = const.tile([1, hid], F32R)
        nc.sync.dma_start(out=bs_t[:], in_=bs_d.ap())
        sdg_t = const.tile([1, BPC * P], F32R)
        nc.sync.dma_start(out=sdg_t[:], in_=sdg_d.ap())
        idr_t = const.tile([P, P], BF16)
        nc.sync.dma_start(out=idr_t[:], in_=idr_d.ap())
        greg = nc.alloc_register(mybir.EngineType.Pool, "gcnt")
        dv_t = const.tile([P, BPC], F32)
        nc.sync.dma_start(out=dv_t[:], in_=dv_d.ap())
        adv_t = const.tile([P, BPC], F32)
        nc.sync.dma_start(out=adv_t[:], in_=adv_d.ap())

        gather_qn = 0
        for g in range(ngroups):
            sg = sched[g]
            blocks = list(range(g * gblk, min((g + 1) * gblk, BPC)))
            nbl = len(blocks)
            # gathers (lo/hi)
            gxs = []
            for h in range(2):
                nidx = sg["nidx"][h]
                gx = gxp.tile(
                    [P, kmax * in_ch], BF16, tag=f"gx{h}", name=f"gx{h}_{g}"
                )
                kk = nidx // P
                soff = (g * 2 + h) * segw
                if not FULLGEN:
                    nc.gpsimd.reg_load(
                        greg, ct_t[0:1, g * 2 + h : g * 2 + h + 1]
                    )
                nc.gpsimd.dma_gather(
                    gx[:, : kk * in_ch].rearrange("p (k d) -> p k d", d=in_ch),
                    x_ds[h].ap(),
                    si_t[:, soff : soff + 8 * kk],
                    nidx,
                    nidx if FULLGEN else greg,
                    in_ch,
                    queue_num=gather_qn % 4,
                    single_packet=False,
                )
                gather_qn += 1
                gxs.append(gx)
            # AT accumulators per (local block, ch half)
            AT = {}
            started = {}
            for bl in range(nbl):
                t = psA.tile([P, nch * P], F32, tag=f"at{bl}", name=f"at{bl}_{g}")
                for hh in range(nch):
                    AT[(bl, hh)] = t[:, hh * P : (hh + 1) * P]
                started[bl] = False
            # msel tiles for this group arrive via one HWDGE DMA
            pc0, pc1 = sg["pc0"], sg["pc1"]
            npg = pc1 - pc0
            ms_t = mselp.tile(
                [P, npg * P], ms_dt, tag="msg", name=f"msg_{g}"
            )
            nc.scalar.dma_start(
                out=ms_t[:], in_=ms_d.ap()[:, pc0 * P : pc1 * P]
            )
            # chunk matmuls: AT_h[c, d] += gx[e, c]^T @ Msel[e, d]
            for h in range(2):
                for (j, bl, col) in sg["mm"][h]:
                    ci = col - pc0
                    gx = gxs[h]
                    for hh in range(nch):
                        nc.tensor.matmul(
                            AT[(bl, hh)],
                            lhsT=gx[:, j * in_ch + hh * P : j * in_ch + (hh + 1) * P],
                            rhs=ms_t[:, ci * P : (ci + 1) * P],
                            start=not started[bl],
                            stop=False,
                        )
                        started[bl] = True
            # per block: self-loop (stop), AT->SBUF, weight mm, PReLU, store
            for bl, b in enumerate(blocks):
                ns = min(P, npc - b * P)
                for hh in range(nch):
                    nc.tensor.matmul(
                        AT[(bl, hh)],
                        lhsT=xs_t[:, b * in_ch + hh * P : b * in_ch + (hh + 1) * P],
                        rhs=idr_t[:],
                        start=not started[bl],
                        stop=(hh == nch - 1),
                    )
                    started[bl] = True
                ats = []
                for hh in range(nch):
                    a = aS.tile([P, P], BF16, tag=f"ats{hh}", name=f"ats{hh}_{b}")
                    nc.scalar.copy(a[:], AT[(bl, hh)])
                    ats.append(a)
                Hp = hps.tile([P, hid], F32, tag="hp", name=f"hp_{b}")
                for hh in range(nch):
                    nc.tensor.matmul(
                        Hp[:ns],
                        lhsT=ats[hh][:, :ns],
                        rhs=wt_t[hh][:],
                        start=(hh == 0),
                        stop=(not has_bias and hh == nch - 1),
                    )
                if has_bias:
                    nc.tensor.matmul(
                        Hp[:ns],
                        lhsT=sdg_t[:, b * P : b * P + ns],
                        rhs=bs_t[:],
                        start=False,
                        stop=True,
                    )
                os_ = outp.tile([P, hid], out_dt, tag="os", name=f"os_{b}")
                if prelu_mode == "act":
                    # PReLU(dinv*H) in one ACT op (exact for any alpha)
                    nc.scalar.activation(
                        out=os_[:ns],
                        in_=Hp[:ns],
                        func=mybir.ActivationFunctionType.Prelu,
                        scale=dv_t[:ns, b : b + 1],
                        alpha=float(alpha),
                    )
                else:
                    # fallback: max(dinv*H, alpha*dinv*H) (0<=alpha<=1)
                    t1 = outp.tile([P, hid], F32, tag="t1", name=f"t1_{b}")
                    t2 = outp.tile([P, hid], F32, tag="t2", name=f"t2_{b}")

                    nc.scalar.activation(
                        out=t1[:ns],
                        in_=Hp[:ns],
                        func=mybir.ActivationFunctionType.Copy,
                        scale=dv_t[:ns, b : b + 1],
                    )
                    nc.scalar.activation(
                        out=t2[:ns],
                        in_=Hp[:ns],
                        func=mybir.ActivationFunctionType.Copy,
                        scale=adv_t[:ns, b : b + 1],
                    )
                    nc.vector.tensor_tensor(
                        out=os_[:ns], in0=t1[:ns], in1=t2[:ns],
                        op=mybir.AluOpType.max,
                    )
                row0 = b * P
                nc.sync.dma_start(
                    out=out_d.ap()[row0 : row0 + ns, :], in_=os_[:ns, :]
                )
    nc.compile()
    return nc


def _make_in_maps(x, weight, bias, idx16, msel, cnts, dinv, alpha, ncores=NCORES):
    x = np.asarray(x, dtype=np.float32)
    w = np.asarray(weight, dtype=np.float32)
    n = x.shape[0]
    half = n // 2
    in_ch = x.shape[1]
    hid = w.shape[0]
    npc = n // ncores
    bf = mybir.dt.np(BF16)

    xp = x * dinv[:, None]  # prescaled by dinv[src]
    xlo = np.ascontiguousarray(xp[:half].astype(bf))
    xhi = np.ascontiguousarray(xp[half:].astype(bf))
    wts = {
        f"wt{h}": np.ascontiguousarray(w[:, h * P : (h + 1) * P].T.astype(bf))
        for h in range(in_ch // P)
    }
    bias_row = np.asarray(bias, dtype=np.float32).reshape(1, hid)
    sdeg = (1.0 / dinv).astype(np.float32)  # sqrt(deg)

    in_maps = []
    for k in range(ncores):
        sl = slice(k * npc, (k + 1) * npc)
        # xsall[d, b*256 + c] = (dinv*x)[k*npc + b*128 + d, c]
        xs = np.zeros((BPC * P, in_ch), np.float32)
        xs[:npc] = xp[sl]
        xsall = np.ascontiguousarray(
            xs.reshape(BPC, P, in_ch).transpose(1, 0, 2).reshape(P, BPC * in_ch)
        ).astype(bf)
        dv = np.zeros((BPC * P,), np.float32)
        dv[:npc] = dinv[sl]
        dvc = np.ascontiguousarray(dv.reshape(BPC, P).T)
        sdg = np.zeros((1, BPC * P), np.float32)
        sdg[0, :npc] = sdeg[sl]
        m = {
            "x0": xlo,
            "x1": xhi,
            "idx16": np.ascontiguousarray(idx16[k]),
            "cnts": np.ascontiguousarray(cnts[k : k + 1]),
            "msel": np.ascontiguousarray(
                msel[k].astype(mybir.dt.np(mybir.dt.float8e4) if MS_DT == "fp8" else bf)
            ),
            "xsall": xsall,
            "bias": bias_row,
            "sdg": sdg,
            "idr": np.eye(P, dtype=bf),
            "dinvc": dvc,
            "adinvc": np.ascontiguousarray(dvc * float(alpha)),
        }
        m.update(wts)
        in_maps.append(m)
    return in_maps


# Results of the last kernel() call, for the test harness.
LAST_RESULTS = None


def kernel(x, edge_index, weight, bias, prelu_a):
    global LAST_RESULTS
    trace = os.environ.get("GCN_TRACE", "0") == "1"

    sched, kmax, npairs, idx16, msel, cnts, dinv = _preprocess(edge_index)
    alpha = float(np.asarray(prelu_a).ravel()[0])
    has_bias = bool(np.any(np.asarray(bias)))
    nc = _build_program(sched, kmax, npairs, alpha, has_bias=has_bias)
    in_maps = _make_in_maps(x, weight, bias, idx16, msel, cnts, dinv, alpha)

    res = bass_utils.run_bass_kernel_spmd(
        nc, in_maps, core_ids=list(range(NCORES)), trace=trace
    )
    LAST_RESULTS = res
    out = np.concatenate(
        [np.asarray(res.results[k]["out"], dtype=np.float32) for k in range(NCORES)],
        axis=0,
    )
    return out
